# revision 24
# baseline (speedup 1.0000x reference)
"""Trainium2 Bass kernel for nn_CustomQuantumLayer (4-qubit circuit, B=1048576).

Algorithm (trig linearization): psi_u = prod_k trig(x_k/2) is, by
product-to-sum identities, linear in T = [sin(theta_s), cos(theta_s)] over the
8 signed half-angle sums theta_s = (x0 +- x1 +- x2 +- x3)/2.  So
phi = U psi = W~ T (one 32x16 real matmul), probs = Re^2 + Im^2, evs = Z probs.

Device pipeline per core (N = 131072 samples, 8 chunks of 512 cols x 32 g):
  M_theta (PE)   x [128=(32g,4k)] -> u = theta/2 duplicated at rows (cs,g',sig)
  frac (DVE/GPS) v = pymod(u + shift[cs], pi)  (shift pi/2 for sin, 3pi/4 cos)
  Sin (ACT)      T = Sin(2v - pi) -> fp16  (= sin theta / cos theta rows)
  M1 (PE)        phi = W~ T, single pass, out [128 = 4smp x 32comp]
  square         sq = phi*phi -> fp16 (split across DVE/ACT/GPS)
  M2 (PE)        evs = Z (sqRe+sqIm), out rows 16/tile into evs psum
  exit + DMA     evs fp32->fp16 -> DRAM; host reorders to [B, 4] fp32.

Weights for M_theta are exact fp16 (+-1/4); range reduction in fp32.
"""
import math

import numpy as np

MAGIC = 12582912.0              # 1.5 * 2**23: fp32 add forces round-to-int
B_TOTAL = 1048576
N_CORES = 8
N_CORE = B_TOTAL // N_CORES     # 131072
COLS = N_CORE // 32             # 4096 sample-columns (32 samples per column)
CHUNK = 512                     # columns per chunk
N_CHUNKS = COLS // CHUNK        # 8

_CACHE = {}

# sigma sign patterns for theta_s = (x0 + s1 x1 + s2 x2 + s3 x3)/2
_SIGMAS = [((i >> 2 & 1) * -2 + 1, (i >> 1 & 1) * -2 + 1, (i & 1) * -2 + 1)
           for i in range(8)]


# ---------------------------------------------------------------- host math
def _build_u(weights):
    w = np.asarray(weights, np.float64)

    def ry(t):
        c, s = np.cos(t / 2), np.sin(t / 2)
        return np.array([[c, -s], [s, c]], np.complex128)

    def rz(t):
        e = np.exp(-0.5j * t)
        return np.array([[e, 0], [0, np.conj(e)]], np.complex128)

    def rot(phi, th, om):
        return rz(om) @ ry(th) @ rz(phi)

    def emb1(g, q):
        m = np.array([[1.0]], np.complex128)
        for k in range(4):
            m = np.kron(m, g if k == q else np.eye(2, dtype=np.complex128))
        return m

    def cnot(c, t):
        m = np.zeros((16, 16), np.complex128)
        for s in range(16):
            bits = [(s >> (3 - k)) & 1 for k in range(4)]
            if bits[c] == 1:
                bits[t] ^= 1
            s2 = sum(b << (3 - k) for k, b in enumerate(bits))
            m[s2, s] = 1.0
        return m

    U = np.eye(16, dtype=np.complex128)
    for l in range(2):
        for q in range(4):
            U = emb1(rot(*w[l, q]), q) @ U
        for q in range(4):
            U = cnot(q, (q + [1, 2][l]) % 4) @ U
    return U


def _build_P():
    """P[16,16]: psi = P @ [cos(theta_s); sin(theta_s)]."""
    P = np.zeros((16, 16))
    for u in range(16):
        bits = [(u >> (3 - k)) & 1 for k in range(4)]  # 1 = sin factor
        n = sum(bits)
        for si, sig in enumerate(_SIGMAS):
            s = 1.0
            for k in range(1, 4):
                if bits[k]:
                    s *= sig[k - 1]
            if n % 2 == 0:
                P[u, si] += s * (-1.0) ** (n // 2) / 8.0        # cos block
            else:
                P[u, 8 + si] += s * (-1.0) ** ((n - 1) // 2) / 8.0  # sin block
    return P


def _build_consts(weights):
    U = _build_u(weights)
    WP = U @ _build_P()                     # phi = WP @ [cos(8); sin(8)]
    Wc = np.concatenate([WP.real, WP.imag], 0)  # [32, 16]

    # M_theta: x rows (4g+k) -> t rows (64cs + 8gl + sig), t = theta/(2 pi)
    # in turns, duplicated for cs in {0 (sin), 1 (cos)}.
    lhsT_th = np.zeros((4, 128, 128))
    for t in range(4):
        for gl in range(8):
            g = 8 * t + gl
            for sig in range(8):
                eps = (1.0,) + _SIGMAS[sig]
                for k in range(4):
                    for cs in range(2):
                        lhsT_th[t, 4 * g + k, 64 * cs + 8 * gl + sig] = \
                            eps[k] / (4 * math.pi)

    # M1: T rows (64cs + 8gl + sig) -> phi rows (16gl + s), one matmul for
    # Re (v=0) and one for Im (v=1) per T-tile; cs=0 rows are sin, cs=1 cos.
    lhsT_m1 = np.zeros((2, 128, 128))
    for v in range(2):
        for gl in range(8):
            for sig in range(8):
                for sc in range(16):
                    o = 16 * gl + sc
                    lhsT_m1[v, 0 + 8 * gl + sig, o] = Wc[16 * v + sc, 8 + sig]
                    lhsT_m1[v, 64 + 8 * gl + sig, o] = Wc[16 * v + sc, sig]

    # M2: probs rows (16gl + sc) -> evs rows (32 sub + 4gl + q); two probs
    # tiles (sub = 0, 1) accumulate into one 64-row psum group (offset 0/64).
    lhsT_z = np.zeros((2, 128, 64))
    for sub in range(2):
        for gl in range(8):
            for sc in range(16):
                for q in range(4):
                    lhsT_z[sub, 16 * gl + sc, 32 * sub + 4 * gl + q] = \
                        1.0 - 2.0 * ((sc >> (3 - q)) & 1)

    # col 0: per-partition phase shift (turns) for the round op; col 1:
    # Sin bias 2 pi s_p; col 2: -MAGIC (strip).  Rows < 64 sin, >= 64 cos.
    # Sign flips from the bias identities cancel in the squares.
    shift = np.empty((128, 4), np.float32)
    shift[:64, 0] = 0.5
    shift[64:, 0] = 0.75
    shift[:64, 1] = 2 * math.pi * 0.5
    shift[64:, 1] = 2 * math.pi * 0.75
    shift[:, 2] = -MAGIC
    shift[:, 3] = 0.0
    negI = -np.eye(128)
    # s_p row for the phase-shift accumulate: out row i gets +1/2 (sin rows,
    # i < 64) or +3/4 (cos rows)
    sp_row = np.empty((1, 128))
    sp_row[0, :64] = 0.5
    sp_row[0, 64:] = 0.75
    return (lhsT_th.astype(np.float16), lhsT_m1.astype(np.float16),
            lhsT_z.astype(np.float16), shift, negI.astype(np.float16),
            sp_row.astype(np.float16))


# ---------------------------------------------------------------- device kernel
# engine assignment for the 8 squares (phi Re/Im per T-tile; "gpsimd" tiles
# are DMA-copied PSUM->SBUF first since GPSIMD cannot read PSUM), the 4
# probs-adds, and the evs exit copy per chunk.
SQ_ENGINES = ["scalar", "vector", "scalar", "vector", "scalar", "vector",
              "scalar", "scalar"]
PADD_ENGINES = ["gpsimd", "gpsimd", "gpsimd", "vector"]
EXIT_ENGINE = "vector"


def _emit_kernel(tc, outs, ins):
    from contextlib import ExitStack

    import concourse.mybir as mybir

    ctx = ExitStack()
    nc = tc.nc
    f32 = mybir.dt.float32
    f16 = mybir.dt.float16
    Act = mybir.ActivationFunctionType
    Alu = mybir.AluOpType
    PI = math.pi

    x_ap = ins["x"]
    out_ap = outs["out"]

    consts = ctx.enter_context(tc.tile_pool(name="consts", bufs=1))
    sb_x = ctx.enter_context(tc.tile_pool(name="x", bufs=2))
    sb_v = ctx.enter_context(tc.tile_pool(name="v", bufs=2))
    sb_t = ctx.enter_context(tc.tile_pool(name="trig", bufs=2))
    sb_sq = ctx.enter_context(tc.tile_pool(name="sq", bufs=10))
    sb_evs = ctx.enter_context(tc.tile_pool(name="evs", bufs=2))
    ps_th = ctx.enter_context(tc.tile_pool(name="ps_th", bufs=4, space="PSUM"))
    ps_phi = ctx.enter_context(tc.tile_pool(name="ps_phi", bufs=2, space="PSUM"))
    ps_evs = ctx.enter_context(tc.tile_pool(name="ps_evs", bufs=2, space="PSUM"))

    lth = consts.tile([128, 4, 128], f16, tag="lth")
    lm1 = consts.tile([128, 2, 128], f16, tag="lm1")
    lz = consts.tile([128, 2, 64], f16, tag="lz")
    lneg = consts.tile([128, 128], f16, tag="lneg")
    lsp = consts.tile([1, 128], f16, tag="lsp")
    ones_t = consts.tile([1, CHUNK], f16, tag="ones")
    shift = consts.tile([128, 4], f32, tag="shift")
    nc.sync.dma_start(out=lth, in_=ins["lhsT_th"].rearrange("t p i -> p t i"))
    nc.sync.dma_start(out=lm1, in_=ins["lhsT_m1"].rearrange("t p i -> p t i"))
    nc.sync.dma_start(out=lz, in_=ins["lhsT_z"].rearrange("s p i -> p s i"))
    nc.sync.dma_start(out=lneg, in_=ins["negI"])
    nc.sync.dma_start(out=lsp, in_=ins["sp_row"])
    nc.sync.dma_start(out=ones_t, in_=ins["ones_row"])
    nc.sync.dma_start(out=shift, in_=ins["shift"])

    for ch in range(N_CHUNKS):
        x_t = sb_x.tile([128, CHUNK], f16, tag="x")
        nc.sync.dma_start(out=x_t, in_=x_ap[:, ch * CHUNK:(ch + 1) * CHUNK])

        mm_t = sb_v.tile([128, 4, CHUNK], f32, tag="mm")
        m16_t = sb_v.tile([128, 4, CHUNK], f16, tag="m16")
        trig = sb_t.tile([128, 4, CHUNK], f16, tag="trig")
        for t in range(4):
            th_ps = ps_th.tile([128, CHUNK], f32, tag="th")
            nc.tensor.matmul(out=th_ps, lhsT=lth[:, t, :], rhs=x_t,
                             start=True, stop=False)
            # carrier = t + s_p + MAGIC (round happens in the fp32 add; DVE)
            nc.vector.tensor_scalar(out=mm_t[:, t, :], in0=th_ps,
                                    scalar1=shift[:, 0:1], scalar2=MAGIC,
                                    op0=Alu.add, op1=Alu.add)
            # m = carrier - MAGIC (exact small int, fp16; SBUF 2x on DVE)
            nc.vector.tensor_scalar(out=m16_t[:, t, :], in0=mm_t[:, t, :],
                                    scalar1=-MAGIC, scalar2=None,
                                    op0=Alu.add)
            # th_ps := t - m  (exact -1 weights, fp32 accumulate)
            nc.tensor.matmul(out=th_ps, lhsT=lneg, rhs=m16_t[:, t, :],
                             start=False, stop=True)
            # T = sin(2 pi (t-m) + 2 pi s_p) = +-sin/cos(theta); signs die
            # in the squares.
            nc.scalar.activation(out=trig[:, t, :], in_=th_ps, func=Act.Sin,
                                 scale=2 * math.pi, bias=shift[:, 1:2])

        evs_ps = ps_evs.tile([128, CHUNK], f32, tag="evs")
        for t in range(4):
            sq_re = None
            for v in range(2):
                phi_ps = ps_phi.tile([128, CHUNK], f32, tag="phi")
                nc.tensor.matmul(out=phi_ps, lhsT=lm1[:, v, :],
                                 rhs=trig[:, t, :], start=True, stop=True)
                sq_t = sb_sq.tile([128, CHUNK], f16, tag="sq")
                eng_name = SQ_ENGINES[2 * t + v]
                if eng_name == "scalar":
                    nc.scalar.activation(out=sq_t, in_=phi_ps, func=Act.Square)
                elif eng_name == "gpsimd":
                    phi_sb = sb_sq.tile([128, CHUNK], f32, tag="phi_sb")
                    nc.sync.dma_start(out=phi_sb, in_=phi_ps)
                    nc.gpsimd.tensor_mul(out=sq_t, in0=phi_sb, in1=phi_sb)
                else:
                    # DVE path: one PSUM read per op and no pow -> cast-copy
                    # to fp16 then dense 2x self-multiply.
                    phi_sb = sb_sq.tile([128, CHUNK], f16, tag="phi_sb")
                    nc.vector.tensor_copy(out=phi_sb, in_=phi_ps)
                    nc.vector.tensor_mul(out=sq_t, in0=phi_sb, in1=phi_sb)
                if v == 0:
                    sq_re = sq_t
                else:
                    probs = sb_sq.tile([128, CHUNK], f16, tag="probs")
                    getattr(nc, PADD_ENGINES[t]).tensor_add(
                        out=probs, in0=sq_re, in1=sq_t)
                    grp, sub = divmod(t, 2)
                    nc.tensor.matmul(out=evs_ps[64 * grp:64 * grp + 64, :],
                                     lhsT=lz[:, sub, :], rhs=probs,
                                     start=(sub == 0), stop=(sub == 1))
        evs_sb = sb_evs.tile([128, CHUNK], f16, tag="evs_sb")
        if EXIT_ENGINE == "scalar":
            nc.scalar.activation(out=evs_sb, in_=evs_ps, func=Act.Copy)
        else:
            nc.vector.tensor_copy(out=evs_sb, in_=evs_ps)
        nc.sync.dma_start(
            out=out_ap[:, ch * CHUNK:(ch + 1) * CHUNK], in_=evs_sb)

    ctx.close()


def _build_program():
    import concourse.bacc as bacc
    import concourse.mybir as mybir
    import concourse.tile as tile

    nc = bacc.Bacc("TRN2", debug=False, num_devices=N_CORES)
    f32 = mybir.dt.float32
    f16 = mybir.dt.float16

    ins = {
        "x": nc.dram_tensor("x", [128, COLS], f16, kind="ExternalInput").ap(),
        "lhsT_th": nc.dram_tensor("lhsT_th", [4, 128, 128], f16,
                                  kind="ExternalInput").ap(),
        "lhsT_m1": nc.dram_tensor("lhsT_m1", [2, 128, 128], f16,
                                  kind="ExternalInput").ap(),
        "lhsT_z": nc.dram_tensor("lhsT_z", [2, 128, 64], f16,
                                 kind="ExternalInput").ap(),
        "shift": nc.dram_tensor("shift", [128, 4], f32,
                                kind="ExternalInput").ap(),
        "negI": nc.dram_tensor("negI", [128, 128], f16,
                               kind="ExternalInput").ap(),
        "sp_row": nc.dram_tensor("sp_row", [1, 128], f16,
                                 kind="ExternalInput").ap(),
        "ones_row": nc.dram_tensor("ones_row", [1, CHUNK], f16,
                                   kind="ExternalInput").ap(),
    }
    outs = {
        "out": nc.dram_tensor("out", [128, COLS], f16,
                              kind="ExternalOutput").ap(),
    }
    with tile.TileContext(nc) as tc:
        _emit_kernel(tc, outs, ins)
    nc.compile()
    return nc


def _get_program():
    if "nc" not in _CACHE:
        _CACHE["nc"] = _build_program()
    return _CACHE["nc"]


def _prep_inputs(x, weights):
    """Full [B,4] fp32 x -> per-core input maps (layout/dtype prep only)."""
    (lhsT_th, lhsT_m1, lhsT_z, shift, negI, sp_row) = _build_consts(weights)
    consts = {"lhsT_th": lhsT_th, "lhsT_m1": lhsT_m1, "lhsT_z": lhsT_z,
              "shift": shift, "negI": negI, "sp_row": sp_row,
              "ones_row": np.ones((1, CHUNK), np.float16)}
    x16 = np.asarray(x, np.float32).astype(np.float16)
    in_maps = []
    for c in range(N_CORES):
        xc = x16[c * N_CORE:(c + 1) * N_CORE]          # [131072, 4]
        xd = np.ascontiguousarray(
            xc.reshape(COLS, 32, 4).transpose(1, 2, 0).reshape(128, COLS))
        m = {"x": xd}
        m.update(consts)
        in_maps.append(m)
    return in_maps


def _decode_out(res):
    """Device outputs [128, COLS] fp16 -> full [B, 4] fp32."""
    parts = []
    for c in range(N_CORES):
        o = np.asarray(res.results[c]["out"])           # [128, 4096] fp16
        # row = 64*grp + 32*sub + 4*gl + q ; sample = 32*col + 16*grp+8*sub+gl
        o = o.reshape(2, 2, 8, 4, COLS)                  # [grp, sub, gl, q, col]
        o = o.transpose(4, 0, 1, 2, 3)                   # [col, grp, sub, gl, q]
        parts.append(o.reshape(N_CORE, 4).astype(np.float32))
    return np.concatenate(parts, axis=0)


def kernel(x: np.ndarray, weights: np.ndarray) -> np.ndarray:
    from concourse import bass_utils

    nc = _get_program()
    in_maps = _prep_inputs(x, weights)
    res = bass_utils.run_bass_kernel_spmd(nc, in_maps,
                                          core_ids=list(range(N_CORES)))
    return _decode_out(res)


# revision 25
# speedup vs baseline: 1.1076x; 1.1076x over previous
"""Trainium2 Bass kernel for nn_CustomQuantumLayer (4-qubit circuit, B=1048576).

Algorithm (trig linearization): psi_u = prod_k trig(x_k/2) is, by
product-to-sum identities, linear in T = [sin(theta_s), cos(theta_s)] over the
8 signed half-angle sums theta_s = (x0 +- x1 +- x2 +- x3)/2.  So
phi = U psi = W~ T (one 32x16 real matmul), probs = Re^2 + Im^2, evs = Z probs.

Device pipeline per core (N = 131072 samples, 8 chunks of 512 cols x 32 g):
  M_theta (PE)   x [128=(32g,4k)] -> u = theta/2 duplicated at rows (cs,g',sig)
  frac (DVE/GPS) v = pymod(u + shift[cs], pi)  (shift pi/2 for sin, 3pi/4 cos)
  Sin (ACT)      T = Sin(2v - pi) -> fp16  (= sin theta / cos theta rows)
  M1 (PE)        phi = W~ T, single pass, out [128 = 4smp x 32comp]
  square         sq = phi*phi -> fp16 (split across DVE/ACT/GPS)
  M2 (PE)        evs = Z (sqRe+sqIm), out rows 16/tile into evs psum
  exit + DMA     evs fp32->fp16 -> DRAM; host reorders to [B, 4] fp32.

Weights for M_theta are exact fp16 (+-1/4); range reduction in fp32.
"""
import math

import numpy as np

MAGIC = 12582912.0              # 1.5 * 2**23: fp32 add forces round-to-int
B_TOTAL = 1048576
N_CORES = 8
N_CORE = B_TOTAL // N_CORES     # 131072
COLS = N_CORE // 32             # 4096 sample-columns (32 samples per column)
CHUNK = 512                     # columns per chunk
N_CHUNKS = COLS // CHUNK        # 8

_CACHE = {}

# sigma sign patterns for theta_s = (x0 + s1 x1 + s2 x2 + s3 x3)/2
_SIGMAS = [((i >> 2 & 1) * -2 + 1, (i >> 1 & 1) * -2 + 1, (i & 1) * -2 + 1)
           for i in range(8)]


# ---------------------------------------------------------------- host math
def _build_u(weights):
    w = np.asarray(weights, np.float64)

    def ry(t):
        c, s = np.cos(t / 2), np.sin(t / 2)
        return np.array([[c, -s], [s, c]], np.complex128)

    def rz(t):
        e = np.exp(-0.5j * t)
        return np.array([[e, 0], [0, np.conj(e)]], np.complex128)

    def rot(phi, th, om):
        return rz(om) @ ry(th) @ rz(phi)

    def emb1(g, q):
        m = np.array([[1.0]], np.complex128)
        for k in range(4):
            m = np.kron(m, g if k == q else np.eye(2, dtype=np.complex128))
        return m

    def cnot(c, t):
        m = np.zeros((16, 16), np.complex128)
        for s in range(16):
            bits = [(s >> (3 - k)) & 1 for k in range(4)]
            if bits[c] == 1:
                bits[t] ^= 1
            s2 = sum(b << (3 - k) for k, b in enumerate(bits))
            m[s2, s] = 1.0
        return m

    U = np.eye(16, dtype=np.complex128)
    for l in range(2):
        for q in range(4):
            U = emb1(rot(*w[l, q]), q) @ U
        for q in range(4):
            U = cnot(q, (q + [1, 2][l]) % 4) @ U
    return U


def _build_P():
    """P[16,16]: psi = P @ [cos(theta_s); sin(theta_s)]."""
    P = np.zeros((16, 16))
    for u in range(16):
        bits = [(u >> (3 - k)) & 1 for k in range(4)]  # 1 = sin factor
        n = sum(bits)
        for si, sig in enumerate(_SIGMAS):
            s = 1.0
            for k in range(1, 4):
                if bits[k]:
                    s *= sig[k - 1]
            if n % 2 == 0:
                P[u, si] += s * (-1.0) ** (n // 2) / 8.0        # cos block
            else:
                P[u, 8 + si] += s * (-1.0) ** ((n - 1) // 2) / 8.0  # sin block
    return P


def _build_consts(weights):
    U = _build_u(weights)
    WP = U @ _build_P()                     # phi = WP @ [cos(8); sin(8)]
    Wc = np.concatenate([WP.real, WP.imag], 0)  # [32, 16]

    # M_theta: x rows (4g+k) -> t rows (64cs + 8gl + sig), t = theta/(2 pi)
    # in turns, duplicated for cs in {0 (sin), 1 (cos)}.
    lhsT_th = np.zeros((4, 128, 128))
    for t in range(4):
        for gl in range(8):
            g = 8 * t + gl
            for sig in range(8):
                eps = (1.0,) + _SIGMAS[sig]
                for k in range(4):
                    for cs in range(2):
                        lhsT_th[t, 4 * g + k, 64 * cs + 8 * gl + sig] = \
                            eps[k] / (4 * math.pi)

    # M1: T rows (64cs + 8gl + sig) -> phi rows (16gl + s), one matmul for
    # Re (v=0) and one for Im (v=1) per T-tile; cs=0 rows are sin, cs=1 cos.
    lhsT_m1 = np.zeros((2, 128, 128))
    for v in range(2):
        for gl in range(8):
            for sig in range(8):
                for sc in range(16):
                    o = 16 * gl + sc
                    lhsT_m1[v, 0 + 8 * gl + sig, o] = Wc[16 * v + sc, 8 + sig]
                    lhsT_m1[v, 64 + 8 * gl + sig, o] = Wc[16 * v + sc, sig]

    # M2: probs rows (16gl + sc) -> evs rows (32 sub + 4gl + q); two probs
    # tiles (sub = 0, 1) accumulate into one 64-row psum group (offset 0/64).
    lhsT_z = np.zeros((2, 128, 64))
    for sub in range(2):
        for gl in range(8):
            for sc in range(16):
                for q in range(4):
                    lhsT_z[sub, 16 * gl + sc, 32 * sub + 4 * gl + q] = \
                        1.0 - 2.0 * ((sc >> (3 - q)) & 1)

    # col 0: per-partition phase shift (turns) for the round op; col 1:
    # Sin bias 2 pi s_p; col 2: -MAGIC (strip).  Rows < 64 sin, >= 64 cos.
    # Sign flips from the bias identities cancel in the squares.
    shift = np.empty((128, 4), np.float32)
    shift[:64, 0] = 0.5
    shift[64:, 0] = 0.75
    shift[:64, 1] = 2 * math.pi * 0.5
    shift[64:, 1] = 2 * math.pi * 0.75
    shift[:, 2] = -MAGIC
    shift[:, 3] = 0.0
    negI = -np.eye(128)
    # s_p row for the phase-shift accumulate: out row i gets +1/2 (sin rows,
    # i < 64) or +3/4 (cos rows)
    sp_row = np.empty((1, 128))
    sp_row[0, :64] = 0.5
    sp_row[0, 64:] = 0.75
    return (lhsT_th.astype(np.float16), lhsT_m1.astype(np.float16),
            lhsT_z.astype(np.float16), shift, negI.astype(np.float16),
            sp_row.astype(np.float16))


# ---------------------------------------------------------------- device kernel
# engine assignment for the 8 squares (phi Re/Im per T-tile; "gpsimd" tiles
# are DMA-copied PSUM->SBUF first since GPSIMD cannot read PSUM), the 4
# probs-adds, and the evs exit copy per chunk.
SQ_ENGINES = ["scalar", "vector", "scalar", "scalar", "scalar", "vector",
              "scalar", "scalar"]
PADD_ENGINES = ["gpsimd", "gpsimd", "gpsimd", "vector"]
EXIT_ENGINE = "vector"


def _emit_kernel(tc, outs, ins):
    from contextlib import ExitStack

    import concourse.mybir as mybir

    ctx = ExitStack()
    nc = tc.nc
    f32 = mybir.dt.float32
    f16 = mybir.dt.float16
    Act = mybir.ActivationFunctionType
    Alu = mybir.AluOpType
    PI = math.pi

    x_ap = ins["x"]
    out_ap = outs["out"]

    consts = ctx.enter_context(tc.tile_pool(name="consts", bufs=1))
    sb_x = ctx.enter_context(tc.tile_pool(name="x", bufs=2))
    sb_v = ctx.enter_context(tc.tile_pool(name="v", bufs=2))
    sb_t = ctx.enter_context(tc.tile_pool(name="trig", bufs=2))
    sb_sq = ctx.enter_context(tc.tile_pool(name="sq", bufs=10))
    sb_evs = ctx.enter_context(tc.tile_pool(name="evs", bufs=2))
    ps_th = ctx.enter_context(tc.tile_pool(name="ps_th", bufs=4, space="PSUM"))
    ps_phi = ctx.enter_context(tc.tile_pool(name="ps_phi", bufs=2, space="PSUM"))
    ps_evs = ctx.enter_context(tc.tile_pool(name="ps_evs", bufs=2, space="PSUM"))

    lth = consts.tile([128, 4, 128], f16, tag="lth")
    lm1 = consts.tile([128, 2, 128], f16, tag="lm1")
    lz = consts.tile([128, 2, 64], f16, tag="lz")
    lneg = consts.tile([128, 128], f16, tag="lneg")
    lsp = consts.tile([1, 128], f16, tag="lsp")
    ones_t = consts.tile([1, CHUNK], f16, tag="ones")
    shift = consts.tile([128, 4], f32, tag="shift")
    nc.sync.dma_start(out=lth, in_=ins["lhsT_th"].rearrange("t p i -> p t i"))
    nc.sync.dma_start(out=lm1, in_=ins["lhsT_m1"].rearrange("t p i -> p t i"))
    nc.sync.dma_start(out=lz, in_=ins["lhsT_z"].rearrange("s p i -> p s i"))
    nc.sync.dma_start(out=lneg, in_=ins["negI"])
    nc.sync.dma_start(out=lsp, in_=ins["sp_row"])
    nc.sync.dma_start(out=ones_t, in_=ins["ones_row"])
    nc.sync.dma_start(out=shift, in_=ins["shift"])

    for ch in range(N_CHUNKS):
        x_t = sb_x.tile([128, CHUNK], f16, tag="x")
        nc.sync.dma_start(out=x_t, in_=x_ap[:, ch * CHUNK:(ch + 1) * CHUNK])

        mm_t = sb_v.tile([128, 4, CHUNK], f32, tag="mm")
        m16_t = sb_v.tile([128, 4, CHUNK], f16, tag="m16")
        trig = sb_t.tile([128, 4, CHUNK], f16, tag="trig")
        for t in range(4):
            th_ps = ps_th.tile([128, CHUNK], f32, tag="th")
            nc.tensor.matmul(out=th_ps, lhsT=lth[:, t, :], rhs=x_t,
                             start=True, stop=False)
            # carrier = t + s_p + MAGIC (round happens in the fp32 add; DVE)
            nc.vector.tensor_scalar(out=mm_t[:, t, :], in0=th_ps,
                                    scalar1=shift[:, 0:1], scalar2=MAGIC,
                                    op0=Alu.add, op1=Alu.add)
            # m = carrier - MAGIC (exact small int, fp16; SBUF 2x on DVE)
            nc.vector.tensor_scalar(out=m16_t[:, t, :], in0=mm_t[:, t, :],
                                    scalar1=-MAGIC, scalar2=None,
                                    op0=Alu.add)
            # th_ps := t - m  (exact -1 weights, fp32 accumulate)
            nc.tensor.matmul(out=th_ps, lhsT=lneg, rhs=m16_t[:, t, :],
                             start=False, stop=True)
            # T = sin(2 pi (t-m) + 2 pi s_p) = +-sin/cos(theta); signs die
            # in the squares.
            nc.scalar.activation(out=trig[:, t, :], in_=th_ps, func=Act.Sin,
                                 scale=2 * math.pi, bias=shift[:, 1:2])

        evs_ps = ps_evs.tile([128, CHUNK], f32, tag="evs")
        for t in range(4):
            sq_re = None
            for v in range(2):
                phi_ps = ps_phi.tile([128, CHUNK], f32, tag="phi")
                nc.tensor.matmul(out=phi_ps, lhsT=lm1[:, v, :],
                                 rhs=trig[:, t, :], start=True, stop=True)
                sq_t = sb_sq.tile([128, CHUNK], f16, tag="sq")
                eng_name = SQ_ENGINES[2 * t + v]
                if eng_name == "scalar":
                    nc.scalar.activation(out=sq_t, in_=phi_ps, func=Act.Square)
                elif eng_name == "gpsimd":
                    phi_sb = sb_sq.tile([128, CHUNK], f32, tag="phi_sb")
                    nc.sync.dma_start(out=phi_sb, in_=phi_ps)
                    nc.gpsimd.tensor_mul(out=sq_t, in0=phi_sb, in1=phi_sb)
                else:
                    # DVE path: one PSUM read per op and no pow -> cast-copy
                    # to fp16 then dense 2x self-multiply.
                    phi_sb = sb_sq.tile([128, CHUNK], f16, tag="phi_sb")
                    nc.vector.tensor_copy(out=phi_sb, in_=phi_ps)
                    nc.vector.tensor_mul(out=sq_t, in0=phi_sb, in1=phi_sb)
                if v == 0:
                    sq_re = sq_t
                else:
                    probs = sb_sq.tile([128, CHUNK], f16, tag="probs")
                    getattr(nc, PADD_ENGINES[t]).tensor_add(
                        out=probs, in0=sq_re, in1=sq_t)
                    grp, sub = divmod(t, 2)
                    nc.tensor.matmul(out=evs_ps[64 * grp:64 * grp + 64, :],
                                     lhsT=lz[:, sub, :], rhs=probs,
                                     start=(sub == 0), stop=(sub == 1))
        evs_sb = sb_evs.tile([128, CHUNK], f16, tag="evs_sb")
        if EXIT_ENGINE == "scalar":
            nc.scalar.activation(out=evs_sb, in_=evs_ps, func=Act.Copy)
        else:
            nc.vector.tensor_copy(out=evs_sb, in_=evs_ps)
        nc.sync.dma_start(
            out=out_ap[:, ch * CHUNK:(ch + 1) * CHUNK], in_=evs_sb)

    ctx.close()


def _build_program():
    import concourse.bacc as bacc
    import concourse.mybir as mybir
    import concourse.tile as tile

    nc = bacc.Bacc("TRN2", debug=False, num_devices=N_CORES)
    f32 = mybir.dt.float32
    f16 = mybir.dt.float16

    ins = {
        "x": nc.dram_tensor("x", [128, COLS], f16, kind="ExternalInput").ap(),
        "lhsT_th": nc.dram_tensor("lhsT_th", [4, 128, 128], f16,
                                  kind="ExternalInput").ap(),
        "lhsT_m1": nc.dram_tensor("lhsT_m1", [2, 128, 128], f16,
                                  kind="ExternalInput").ap(),
        "lhsT_z": nc.dram_tensor("lhsT_z", [2, 128, 64], f16,
                                 kind="ExternalInput").ap(),
        "shift": nc.dram_tensor("shift", [128, 4], f32,
                                kind="ExternalInput").ap(),
        "negI": nc.dram_tensor("negI", [128, 128], f16,
                               kind="ExternalInput").ap(),
        "sp_row": nc.dram_tensor("sp_row", [1, 128], f16,
                                 kind="ExternalInput").ap(),
        "ones_row": nc.dram_tensor("ones_row", [1, CHUNK], f16,
                                   kind="ExternalInput").ap(),
    }
    outs = {
        "out": nc.dram_tensor("out", [128, COLS], f16,
                              kind="ExternalOutput").ap(),
    }
    with tile.TileContext(nc) as tc:
        _emit_kernel(tc, outs, ins)
    nc.compile()
    return nc


def _get_program():
    if "nc" not in _CACHE:
        _CACHE["nc"] = _build_program()
    return _CACHE["nc"]


def _prep_inputs(x, weights):
    """Full [B,4] fp32 x -> per-core input maps (layout/dtype prep only)."""
    (lhsT_th, lhsT_m1, lhsT_z, shift, negI, sp_row) = _build_consts(weights)
    consts = {"lhsT_th": lhsT_th, "lhsT_m1": lhsT_m1, "lhsT_z": lhsT_z,
              "shift": shift, "negI": negI, "sp_row": sp_row,
              "ones_row": np.ones((1, CHUNK), np.float16)}
    x16 = np.asarray(x, np.float32).astype(np.float16)
    in_maps = []
    for c in range(N_CORES):
        xc = x16[c * N_CORE:(c + 1) * N_CORE]          # [131072, 4]
        xd = np.ascontiguousarray(
            xc.reshape(COLS, 32, 4).transpose(1, 2, 0).reshape(128, COLS))
        m = {"x": xd}
        m.update(consts)
        in_maps.append(m)
    return in_maps


def _decode_out(res):
    """Device outputs [128, COLS] fp16 -> full [B, 4] fp32."""
    parts = []
    for c in range(N_CORES):
        o = np.asarray(res.results[c]["out"])           # [128, 4096] fp16
        # row = 64*grp + 32*sub + 4*gl + q ; sample = 32*col + 16*grp+8*sub+gl
        o = o.reshape(2, 2, 8, 4, COLS)                  # [grp, sub, gl, q, col]
        o = o.transpose(4, 0, 1, 2, 3)                   # [col, grp, sub, gl, q]
        parts.append(o.reshape(N_CORE, 4).astype(np.float32))
    return np.concatenate(parts, axis=0)


def kernel(x: np.ndarray, weights: np.ndarray) -> np.ndarray:
    from concourse import bass_utils

    nc = _get_program()
    in_maps = _prep_inputs(x, weights)
    res = bass_utils.run_bass_kernel_spmd(nc, in_maps,
                                          core_ids=list(range(N_CORES)))
    return _decode_out(res)
